# revision 32
# baseline (speedup 1.0000x reference)
"""AIMv2 multi-head attention (B=4, N=2048, C=1024, H=8) on 8 TRN2 NeuronCores.

Sharding: (batch x token-half) -> 8 shards, one per core. Each core:
  - computes q for its 1024 query tokens (all heads),
  - redundantly computes k / v^T for its batch's full 2048 tokens
    (cheaper than any cross-core collective at these sizes),
  - does softmax(q k^T / sqrt(D)) v for all 8 heads with scores built
    TRANSPOSED ([k_t, q_t]) so the PE contracts over the partition dim in
    both matmuls without any on-chip transposes,
  - applies the output projection and writes its [1024, 1024] slice.
No collectives. Weights/x are pre-transposed and cast to bf16 on the host
so every matmul operand is in its natural [K(part), M/N(free)] layout.

The per-head stream is software-pipelined BY EMISSION ORDER (engine streams
execute in order): each kt step emits S^T(h,kt) -> exp -> PV(h,kt) plus a
chunk of k(h+1), so the PE stays busy under the ACT exp windows; the softmax
denominator runs as a DVE/GpSimd accumulate chain alongside.
"""

import sys

sys.path.insert(0, "/opt/trn_rl_repo")

import numpy as np
import ml_dtypes

import concourse.bass as bass
import concourse.mybir as mybir
import concourse.tile as tile
from concourse import bacc
from concourse.bass_utils import run_bass_kernel_spmd

B, N, C, H, D = 4, 2048, 1024, 8, 128
NQ = N // 2          # query tokens per core
CB = C // 128        # contraction blocks
KT = N // 128        # key-token 128-blocks
NPT = 4              # rotating P^T buffers
import os as _os
PV_LAG = _os.environ.get('PV_LAG', '0') == '1'
BF = mybir.dt.bfloat16
F32 = mybir.dt.float32
F32R = mybir.dt.float32r
SCALE = float(1.0 / np.sqrt(D))
AF = mybir.ActivationFunctionType
ALU = mybir.AluOpType


def _emit(nc, tc, pools, params, r):
    """Emit one full forward pass. r = rep index (benchmarking only)."""
    wp, xp, qkvp, ppool, misc, psA, psB, psO = pools
    xT, xqT, WqkvT, WprojT, out = params
    ones, ones_row = pools.ones, pools.ones_row

    q_sb = [qkvp.tile([128, NQ], BF, tag=f"q{h}", name=f"r{r}q{h}") for h in range(H)]
    v_sb = [qkvp.tile([128, C], BF, tag=f"v{kt}", name=f"r{r}v{kt}") for kt in range(KT)]
    a_sb = [qkvp.tile([128, NQ], BF, tag=f"a{h}", name=f"r{r}a{h}") for h in range(H)]

    # ---- q pass: q[o, t] for own-half tokens; W stationary, x moving -----
    # (small q DMAs first so the PE starts ASAP; big resident-x DMAs follow)
    wq = [wp.tile([128, C], BF, tag=f"w{c}", name=f"r{r}wq{c}") for c in range(CB)]
    xq = [xp.tile([128, NQ], BF, tag=f"x{c}", bufs=1, name=f"r{r}xq{c}") for c in range(CB)]
    for c in range(CB):
        nc.sync.dma_start(out=wq[c], in_=WqkvT[c * 128:(c + 1) * 128, 0:C])
        nc.sync.dma_start(out=xq[c], in_=xqT[c * 128:(c + 1) * 128, :])

    # resident x^T (bf16): all k/v matmuls read it from SBUF
    xr = [qkvp.tile([128, N], BF, tag=f"xr{c}", name=f"r{r}xr{c}") for c in range(CB)]
    for c in range(CB):
        nc.sync.dma_start(out=xr[c], in_=xT[c * 128:(c + 1) * 128, :])
    for t2 in range(NQ // 512):
        for h in range(H):
            ps = psB.tile([128, 512], F32, tag="b", name=f"r{r}psq{t2}_{h}")
            for c in range(CB):
                nc.tensor.matmul(ps, lhsT=wq[c][:, h * 128:(h + 1) * 128],
                                 rhs=xq[c][:, t2 * 512:(t2 + 1) * 512],
                                 start=(c == 0), stop=(c == CB - 1))
            if h % 2 == 0:
                nc.scalar.copy(out=q_sb[h][:, t2 * 512:(t2 + 1) * 512], in_=ps)
            else:
                nc.vector.tensor_copy(out=q_sb[h][:, t2 * 512:(t2 + 1) * 512], in_=ps)

    # ---- v: computed chunk-wise inside head 0's stream (see below) -------
    wv = [wp.tile([128, C], BF, tag=f"w{c}", name=f"r{r}wv{c}") for c in range(CB)]
    for c in range(CB):
        nc.sync.dma_start(out=wv[c], in_=WqkvT[c * 128:(c + 1) * 128, 2 * C:3 * C])

    def emit_v_chunk(kt):
        for o2 in range(2):
            ps = psB.tile([128, 512], F32, tag="b", name=f"r{r}psv{kt}_{o2}")
            for c in range(CB):
                nc.tensor.matmul(ps, lhsT=xr[c][:, kt * 128:(kt + 1) * 128],
                                 rhs=wv[c][:, o2 * 512:(o2 + 1) * 512],
                                 start=(c == 0), stop=(c == CB - 1))
            nc.scalar.copy(out=v_sb[kt][:, o2 * 512:(o2 + 1) * 512], in_=ps)

    # ---- k for head 0 (prologue) ----------------------------------------
    wk = [wp.tile([128, C], BF, tag=f"w{c}", name=f"r{r}wk{c}") for c in range(CB)]
    for c in range(CB):
        nc.sync.dma_start(out=wk[c], in_=WqkvT[c * 128:(c + 1) * 128, C:2 * C])

    def emit_k_chunk(h, t2, kh_tile):
        ps = psB.tile([128, 512], F32, tag="b", name=f"r{r}psk{h}_{t2}")
        for c in range(CB):
            nc.tensor.matmul(ps, lhsT=wk[c][:, h * 128:(h + 1) * 128],
                             rhs=xr[c][:, t2 * 512:(t2 + 1) * 512],
                             start=(c == 0), stop=(c == CB - 1))
        nc.vector.tensor_copy(out=kh_tile[:, t2 * 512:(t2 + 1) * 512], in_=ps)

    kh = [None, None]
    kh[0] = qkvp.tile([128, N], BF, tag="kh0", name=f"r{r}k0")
    for t2 in range(N // 512):
        emit_k_chunk(0, t2, kh[0])

    # ---- per-head interleaved stream ------------------------------------
    for h in range(H):
        cur = kh[h % 2]
        if h + 1 < H:
            kh[(h + 1) % 2] = qkvp.tile([128, N], BF, tag=f"kh{(h + 1) % 2}",
                                        name=f"r{r}k{h + 1}")
        nxt = kh[(h + 1) % 2]

        pts = [None] * NPT
        acc_e = misc.tile([128, NQ], F32R, tag="acc0", bufs=1, name=f"r{r}acce{h}")
        acc_o = misc.tile([128, NQ], F32R, tag="acc1", bufs=1, name=f"r{r}acco{h}")
        ops = psO.tile([128, NQ], F32, tag="o", name=f"r{r}ov{h}")
        for kt in range(KT):
            if h == 0:
                emit_v_chunk(kt)
            # scores S^T[kt-block, q] and exp
            sps = psA.tile([128, NQ], F32, tag="s", name=f"r{r}s{h}_{kt}")
            for q2 in range(NQ // 512):
                nc.tensor.matmul(sps[:, q2 * 512:(q2 + 1) * 512],
                                 lhsT=cur[:, kt * 128:(kt + 1) * 128],
                                 rhs=q_sb[h][:, q2 * 512:(q2 + 1) * 512],
                                 start=True, stop=True)
            pt = ppool.tile([128, NQ], BF, tag=f"p{kt % NPT}", name=f"r{r}p{h}_{kt}")
            pts[kt % NPT] = pt
            nc.scalar.activation(pt, sps, AF.Exp, scale=SCALE)

            # PV lags one kt behind exp (PV_LAG=1) so the PE never waits on a
            # fresh cross-engine result, or runs immediately (PV_LAG=0)
            if PV_LAG:
                if kt >= 1:
                    pv = kt - 1
                    for q2 in range(NQ // 512):
                        nc.tensor.matmul(ops[:, q2 * 512:(q2 + 1) * 512],
                                         lhsT=v_sb[pv][:, h * 128:(h + 1) * 128],
                                         rhs=pts[pv % NPT][:, q2 * 512:(q2 + 1) * 512],
                                         start=(pv == 0), stop=False)
            else:
                for q2 in range(NQ // 512):
                    nc.tensor.matmul(ops[:, q2 * 512:(q2 + 1) * 512],
                                     lhsT=v_sb[kt][:, h * 128:(h + 1) * 128],
                                     rhs=pt[:, q2 * 512:(q2 + 1) * 512],
                                     start=(kt == 0), stop=(kt == KT - 1))

            # denominator: two parallel accumulate chains (even kt on DVE,
            # odd kt on GpSimd); the den matmul PSUM-accumulates both
            if kt == 2:
                nc.vector.tensor_tensor(out=acc_e, in0=pts[0], in1=pt, op=ALU.add)
            elif kt == 3:
                nc.gpsimd.tensor_tensor(out=acc_o, in0=pts[1], in1=pt, op=ALU.add)
            elif kt >= 4 and kt % 2 == 0:
                nc.vector.tensor_tensor(out=acc_e, in0=acc_e, in1=pt, op=ALU.add)
            elif kt >= 5 and kt % 2 == 1:
                nc.gpsimd.tensor_tensor(out=acc_o, in0=acc_o, in1=pt, op=ALU.add)

            # k for the next head, one quarter every 4 kt steps
            if h + 1 < H and kt % 4 == 3:
                emit_k_chunk(h + 1, kt // 4, nxt)

        # last PV step closes the accumulation group
        if PV_LAG:
            for q2 in range(NQ // 512):
                nc.tensor.matmul(ops[:, q2 * 512:(q2 + 1) * 512],
                                 lhsT=v_sb[KT - 1][:, h * 128:(h + 1) * 128],
                                 rhs=pts[(KT - 1) % NPT][:, q2 * 512:(q2 + 1) * 512],
                                 start=False, stop=True)

        # tail: denominator -> recip -> broadcast -> normalize
        recip = misc.tile([1, NQ], F32R, tag="recip", bufs=2, name=f"r{r}recip{h}")
        for q2 in range(NQ // 512):
            den = psB.tile([128, 512], F32, tag="b", name=f"r{r}den{h}_{q2}")
            nc.tensor.matmul(den[0:1, :], lhsT=ones,
                             rhs=acc_e[:, q2 * 512:(q2 + 1) * 512],
                             start=True, stop=False)
            nc.tensor.matmul(den[0:1, :], lhsT=ones,
                             rhs=acc_o[:, q2 * 512:(q2 + 1) * 512],
                             start=False, stop=True)
            with nc.allow_low_precision(reason="recip rounded to f32r for the broadcast matmul"):
                nc.vector.reciprocal(recip[:, q2 * 512:(q2 + 1) * 512], den[0:1, :])
        for q2 in range(NQ // 512):
            bc = psB.tile([128, 512], F32, tag="b", name=f"r{r}bc{h}_{q2}")
            nc.tensor.matmul(bc, lhsT=ones_row,
                             rhs=recip[:, q2 * 512:(q2 + 1) * 512],
                             start=True, stop=True)
            bcs = misc.tile([128, 512], F32, tag="bcs", name=f"r{r}bcs{h}_{q2}")
            nc.scalar.copy(out=bcs, in_=bc)
            nc.vector.tensor_tensor(out=a_sb[h][:, q2 * 512:(q2 + 1) * 512],
                                    in0=ops[:, q2 * 512:(q2 + 1) * 512],
                                    in1=bcs, op=ALU.mult)

    # ---- output projection ----------------------------------------------
    wpj = [wp.tile([128, C], BF, tag=f"w{c}", name=f"r{r}wpj{c}") for c in range(CB)]
    for c in range(CB):
        nc.sync.dma_start(out=wpj[c], in_=WprojT[c * 128:(c + 1) * 128, :])
    for t in range(NQ // 128):
        for o2 in range(2):
            ps = psB.tile([128, 512], F32, tag="b", name=f"r{r}pp{t}_{o2}")
            for c in range(CB):
                nc.tensor.matmul(ps, lhsT=a_sb[c][:, t * 128:(t + 1) * 128],
                                 rhs=wpj[c][:, o2 * 512:(o2 + 1) * 512],
                                 start=(c == 0), stop=(c == CB - 1))
            stg = misc.tile([128, 512], F32, tag="ostg", name=f"r{r}stg{t}_{o2}")
            nc.scalar.copy(out=stg, in_=ps)
            nc.sync.dma_start(out=out[t * 128:(t + 1) * 128, o2 * 512:(o2 + 1) * 512], in_=stg)


class _Pools(tuple):
    pass


def build_bass(reps: int = 1) -> bass.Bass:
    nc = bacc.Bacc("TRN2", target_bir_lowering=False, debug=False, num_devices=8)
    xT = nc.declare_dram_parameter("xT", [C, N], BF, isOutput=False)
    xqT = nc.declare_dram_parameter("xqT", [C, NQ], BF, isOutput=False)
    WqkvT = nc.declare_dram_parameter("WqkvT", [C, 3 * C], BF, isOutput=False)
    WprojT = nc.declare_dram_parameter("WprojT", [C, C], BF, isOutput=False)
    out = nc.declare_dram_parameter("out", [NQ, C], F32, isOutput=True)
    params = (xT, xqT, WqkvT, WprojT, out)

    with tile.TileContext(nc) as tc:
        with (
            tc.tile_pool(name="wp", bufs=2) as wp,        # pass weights
            tc.tile_pool(name="xp", bufs=1) as xp,        # xq tiles
            tc.tile_pool(name="qkv", bufs=1) as qkvp,     # resident x/q/vT/attn/k
            tc.tile_pool(name="pp", bufs=1) as ppool,     # exp'd probs P^T (rotating)
            tc.tile_pool(name="misc", bufs=2) as misc,
            tc.tile_pool(name="psA", bufs=2, space="PSUM") as psA,  # scores [128,1024]
            tc.tile_pool(name="psB", bufs=2, space="PSUM") as psB,  # [128,512] qkv/proj/den
            tc.tile_pool(name="psO", bufs=1, space="PSUM") as psO,  # PV out [128,1024]
        ):
            pools = _Pools((wp, xp, qkvp, ppool, misc, psA, psB, psO))
            ones32 = misc.tile([128, 1], F32, tag="ones32", bufs=1, name="ones32")
            nc.vector.memset(ones32, 1.0)
            ones = misc.tile([128, 1], F32R, tag="ones", bufs=1, name="ones")
            nc.vector.tensor_copy(out=ones, in_=ones32)
            ones_row32 = misc.tile([1, 128], F32, tag="ones_row32", bufs=1, name="ones_row32")
            nc.vector.memset(ones_row32, 1.0)
            ones_row = misc.tile([1, 128], F32R, tag="ones_row", bufs=1, name="ones_row")
            nc.vector.tensor_copy(out=ones_row, in_=ones_row32)
            pools.ones, pools.ones_row = ones, ones_row
            for r in range(reps):
                _emit(nc, tc, pools, params, r)
    nc.compile()
    return nc


_NC_CACHE = {}


def _get_nc(reps: int = 1):
    if reps not in _NC_CACHE:
        _NC_CACHE[reps] = build_bass(reps)
    return _NC_CACHE[reps]


def _make_in_maps(x, Wqkv, Wproj):
    bf = ml_dtypes.bfloat16
    WqkvT = np.ascontiguousarray(Wqkv.T).astype(bf)
    WprojT = np.ascontiguousarray(Wproj.T).astype(bf)
    in_maps = []
    for core in range(8):
        b, half = core // 2, core % 2
        xT_b = np.ascontiguousarray(x[b].T).astype(bf)
        xqT = np.ascontiguousarray(x[b, half * NQ:(half + 1) * NQ].T).astype(bf)
        in_maps.append({"xT": xT_b, "xqT": xqT, "WqkvT": WqkvT, "WprojT": WprojT})
    return in_maps


def _assemble(results):
    out = np.empty((B, N, C), np.float32)
    for core in range(8):
        b, half = core // 2, core % 2
        out[b, half * NQ:(half + 1) * NQ] = results[core]["out"]
    return out


def run_reps(x, Wqkv, Wproj, reps: int = 1):
    """Benchmarking entry: same kernel body emitted `reps` times in one NEFF."""
    res = run_bass_kernel_spmd(_get_nc(reps), _make_in_maps(x, Wqkv, Wproj),
                               core_ids=list(range(8)))
    return _assemble(res.results), res


def kernel(x, Wqkv, Wproj):
    res = run_bass_kernel_spmd(_get_nc(1), _make_in_maps(x, Wqkv, Wproj),
                               core_ids=list(range(8)))
    return _assemble(res.results)


if __name__ == "__main__":
    nc = build_bass()
    print("built ok")
